# revision 7
# baseline (speedup 1.0000x reference)
"""CopyOnlyGenerator kernel for 8 TRN2 NeuronCores.

Data-parallel over batch: core c handles batches [4c, 4c+4).
Per batch: switch probs p = softmax(hiddens @ W.T + b); the two one-hot
scatter einsums run as dense TensorE matmuls. Precision trick: the upper
two bytes of an f32 SBUF tile are read in place as a truncated-bf16
"hi" operand (stride-2 bf16 view, no conversion pass); a single DVE sub
produces the bf16 "lo" residual; hi+lo matmuls accumulate in the same
PSUM bank, recovering ~2^-17 relative precision at full bf16 PE rate.
The one-hot maps are exact in bf16, so their stride-2 views are exact.
p scaling is folded into the PSUM->SBUF copies on ScalarE; argmax via
DVE max/max_index.
"""
import sys

for _p in ("/opt/trn_rl_repo", "/root/.axon_site/_ro/trn_rl_repo"):
    if _p not in sys.path:
        sys.path.append(_p)

import numpy as np
import concourse.bass as bass
import concourse.bacc as bacc
import concourse.mybir as mybir
import concourse.tile as tile
from concourse.bass_utils import run_bass_kernel_spmd

f32 = mybir.dt.float32
bf16 = mybir.dt.bfloat16
i32 = mybir.dt.int32
u32 = mybir.dt.uint32
AL = mybir.AluOpType
AF = mybir.ActivationFunctionType

B, T, S, H = 32, 512, 512, 1024
DVS, DVT = 2048, 1024
NV = 1 + DVS + DVT  # 3073
N_CORES = 8
BPC = B // N_CORES  # batches per core


def _hi(ap):
    """Stride-2 bf16 view selecting the upper 2 bytes of each f32 element."""
    v = ap.bitcast(bf16)
    return v.rearrange("p a (n two) -> p a n two", two=2)[:, :, :, 1]


def _build():
    nc = bacc.Bacc(None, target_bir_lowering=False)
    # inputs are host-retiled to [b, partition, k, free] so each DMA row is
    # one fat contiguous read per partition
    hid_d = nc.declare_dram_parameter("hid", [BPC, 128, 4, H], f32, isOutput=False)
    ast_d = nc.declare_dram_parameter("ast", [BPC, 128, 4, T], f32, isOutput=False)
    att_d = nc.declare_dram_parameter("att", [BPC, 128, 4, T], f32, isOutput=False)
    ms_d = nc.declare_dram_parameter("ms", [BPC, 128, 4, DVS], f32, isOutput=False)
    mt_d = nc.declare_dram_parameter("mt", [BPC, 128, 4, DVT], f32, isOutput=False)
    w_d = nc.declare_dram_parameter("wb", [128, 3, H], f32, isOutput=False)
    b_d = nc.declare_dram_parameter("bb", [128, 3], f32, isOutput=False)
    probs_d = nc.declare_dram_parameter("probs", [BPC, T, NV], f32, isOutput=True)
    preds_d = nc.declare_dram_parameter("preds", [BPC, T], i32, isOutput=True)

    with tile.TileContext(nc) as tc:
        with tc.tile_pool(name="sbuf", bufs=1) as cpool, \
             tc.tile_pool(name="maps", bufs=2) as mpool, \
             tc.tile_pool(name="attn", bufs=2) as apool, \
             tc.tile_pool(name="work", bufs=2) as wpool, \
             tc.tile_pool(name="psum", bufs=8, space="PSUM") as psum:
            w_sb = cpool.tile([128, 3, H], f32, tag="w")
            b_sb = cpool.tile([128, 3], f32, tag="b")
            nc.sync.dma_start(w_sb[:], w_d[:])
            nc.sync.dma_start(b_sb[:], b_d[:])

            for b in range(BPC):
                # ---- batch input loads ----
                ms_sb = mpool.tile([128, 4, DVS], f32, tag="ms")
                mt_sb = mpool.tile([128, 4, DVT], f32, tag="mt")
                nc.sync.dma_start(ms_sb[:], ms_d[b])
                nc.sync.dma_start(mt_sb[:], mt_d[b])
                ast_sb = apool.tile([128, 4, T], f32, tag="ast")
                att_sb = apool.tile([128, 4, T], f32, tag="att")
                nc.sync.dma_start(ast_sb[:], ast_d[b])
                nc.sync.dma_start(att_sb[:], att_d[b])

                # hi = in-place bf16 view; lo = bf16(x - hi), one DVE op each
                as_lo = apool.tile([128, 4, T], bf16, tag="as_lo")
                at_lo = apool.tile([128, 4, T], bf16, tag="at_lo")
                nc.gpsimd.tensor_tensor(out=as_lo[:], in0=ast_sb[:], in1=_hi(ast_sb[:]),
                                        op=AL.subtract)
                nc.gpsimd.tensor_tensor(out=at_lo[:], in0=att_sb[:], in1=_hi(att_sb[:]),
                                        op=AL.subtract)
                as_hi, at_hi = _hi(ast_sb[:]), _hi(att_sb[:])
                ms_hi, mt_hi = _hi(ms_sb[:]), _hi(mt_sb[:])

                hid_sb = cpool.tile([128, 4, H], f32, tag="hid")
                nc.sync.dma_start(hid_sb[:], hid_d[b])
                for ti in range(4):
                    tsl = slice(ti * 128, (ti + 1) * 128)
                    # ---- logits (GpSimd) + softmax ----
                    scratch = cpool.tile([128, H], f32, tag="scratch")
                    logits = wpool.tile([128, 3], f32, tag="logits")
                    for k in range(3):
                        nc.vector.scalar_tensor_tensor(
                            scratch[:], hid_sb[:, ti, :], 1.0, w_sb[:, k, :],
                            AL.mult, AL.mult, accum_out=logits[:, k:k + 1])
                    lb = wpool.tile([128, 3], f32, tag="lb")
                    nc.vector.tensor_add(lb[:], logits[:], b_sb[:])
                    mx = wpool.tile([128, 1], f32, tag="mx")
                    nc.vector.reduce_max(mx[:], lb[:], axis=mybir.AxisListType.X)
                    negmx = wpool.tile([128, 1], f32, tag="negmx")
                    nc.vector.tensor_scalar_mul(negmx[:], mx[:], -1.0)
                    e3 = wpool.tile([128, 3], f32, tag="e3")
                    se = wpool.tile([128, 1], f32, tag="se")
                    nc.scalar.activation(e3[:], lb[:], AF.Exp, bias=negmx[:],
                                         scale=1.0, accum_out=se[:])
                    rse = wpool.tile([128, 1], f32, tag="rse")
                    nc.vector.reciprocal(rse[:], se[:])
                    p3 = wpool.tile([128, 3], f32, tag="p3")
                    nc.vector.tensor_scalar_mul(p3[:], e3[:], rse[:, 0:1])

                    # ---- matmuls + assembly ----
                    probs_sb = wpool.tile([128, NV], f32, tag="probs")
                    nc.scalar.copy(probs_sb[:, 0:1], p3[:, 2:3])
                    for v in range(4):
                        ps = psum.tile([128, 512], f32, tag="ps")
                        for k in range(4):
                            nc.tensor.matmul(ps[:], as_hi[:, k, tsl],
                                             ms_hi[:, k, v * 512:(v + 1) * 512],
                                             start=(k == 0), stop=False)
                            nc.tensor.matmul(ps[:], as_lo[:, k, tsl],
                                             ms_hi[:, k, v * 512:(v + 1) * 512],
                                             start=False, stop=(k == 3))
                        nc.scalar.mul(probs_sb[:, 1 + v * 512:1 + (v + 1) * 512],
                                      ps[:], p3[:, 0:1])
                    for v in range(2):
                        ps = psum.tile([128, 512], f32, tag="ps")
                        for k in range(4):
                            nc.tensor.matmul(ps[:], at_hi[:, k, tsl],
                                             mt_hi[:, k, v * 512:(v + 1) * 512],
                                             start=(k == 0), stop=False)
                            nc.tensor.matmul(ps[:], at_lo[:, k, tsl],
                                             mt_hi[:, k, v * 512:(v + 1) * 512],
                                             start=False, stop=(k == 3))
                        nc.scalar.mul(probs_sb[:, 1 + DVS + v * 512:1 + DVS + (v + 1) * 512],
                                      ps[:], p3[:, 1:2])

                    # ---- argmax ----
                    max8 = wpool.tile([128, 8], f32, tag="max8")
                    idx8 = wpool.tile([128, 8], u32, tag="idx8")
                    nc.vector.max(max8[:], probs_sb[:])
                    nc.vector.max_index(idx8[:], max8[:], probs_sb[:])

                    # ---- stores ----
                    nc.scalar.dma_start(probs_d[b, tsl, :], probs_sb[:])
                    nc.scalar.dma_start(preds_d[b, tsl].bitcast(u32), idx8[:, 0:1])

    nc.compile()
    return nc


_NC = None


def kernel(hiddens, source_attentions, source_attention_maps,
           target_attentions, target_attention_maps, W, b):
    global _NC
    if _NC is None:
        _NC = _build()
    nc = _NC

    hiddens = np.asarray(hiddens, dtype=np.float32)
    As = np.asarray(source_attentions, dtype=np.float32)
    At = np.asarray(target_attentions, dtype=np.float32)
    Ms = np.asarray(source_attention_maps, dtype=np.float32)
    Mt = np.asarray(target_attention_maps, dtype=np.float32)
    W = np.asarray(W, dtype=np.float32)
    b = np.asarray(b, dtype=np.float32)

    def tile4(x):  # [B, 4*128, V] -> [B, 128, 4, V]
        Bn, R, V = x.shape
        return np.ascontiguousarray(x.reshape(Bn, 4, 128, V).transpose(0, 2, 1, 3))

    ast = tile4(As.transpose(0, 2, 1))
    att = tile4(At.transpose(0, 2, 1))
    hiddens = tile4(hiddens)
    Ms = tile4(Ms)
    Mt = tile4(Mt)
    wb = np.ascontiguousarray(np.broadcast_to(W[None], (128, 3, H)))
    bb = np.ascontiguousarray(np.broadcast_to(b[None], (128, 3)))

    in_maps = []
    for c in range(N_CORES):
        sl = slice(c * BPC, (c + 1) * BPC)
        in_maps.append({
            "hid": hiddens[sl], "ast": ast[sl], "att": att[sl],
            "ms": Ms[sl], "mt": Mt[sl], "wb": wb, "bb": bb,
        })

    res = run_bass_kernel_spmd(nc, in_maps, core_ids=list(range(N_CORES)))
    probs = np.concatenate([res.results[c]["probs"] for c in range(N_CORES)], axis=0)
    preds = np.concatenate([res.results[c]["preds"] for c in range(N_CORES)], axis=0)
    return probs, preds.astype(np.int32)


# revision 8
# speedup vs baseline: 1.2334x; 1.2334x over previous
"""CopyOnlyGenerator kernel for 8 TRN2 NeuronCores.

Data-parallel over batch: core c handles batches [4c, 4c+4).
Per batch: switch probs p = softmax(hiddens @ W.T + b); the two one-hot
scatter einsums run as dense TensorE matmuls. Precision trick: the upper
two bytes of an f32 SBUF tile are read in place as a truncated-bf16
"hi" operand (stride-2 bf16 view, no conversion pass); a single DVE sub
produces the bf16 "lo" residual; hi+lo matmuls accumulate in the same
PSUM bank, recovering ~2^-17 relative precision at full bf16 PE rate.
The one-hot maps are exact in bf16, so their stride-2 views are exact.
p scaling is folded into the PSUM->SBUF copies on ScalarE; argmax via
DVE max/max_index.
"""
import sys

for _p in ("/opt/trn_rl_repo", "/root/.axon_site/_ro/trn_rl_repo"):
    if _p not in sys.path:
        sys.path.append(_p)

import numpy as np
import ml_dtypes
import concourse.bass as bass
import concourse.bacc as bacc
import concourse.mybir as mybir
import concourse.tile as tile
from concourse.bass_utils import run_bass_kernel_spmd

f32 = mybir.dt.float32
bf16 = mybir.dt.bfloat16
i32 = mybir.dt.int32
u32 = mybir.dt.uint32
AL = mybir.AluOpType
AF = mybir.ActivationFunctionType

B, T, S, H = 32, 512, 512, 1024
DVS, DVT = 2048, 1024
NV = 1 + DVS + DVT  # 3073
N_CORES = 8
BPC = B // N_CORES  # batches per core


def _hi(ap):
    """Stride-2 bf16 view selecting the upper 2 bytes of each f32 element."""
    v = ap.bitcast(bf16)
    return v.rearrange("p a (n two) -> p a n two", two=2)[:, :, :, 1]


def _build():
    nc = bacc.Bacc(None, target_bir_lowering=False)
    # inputs are host-retiled to [b, partition, k, free] so each DMA row is
    # one fat contiguous read per partition
    hid_d = nc.declare_dram_parameter("hid", [BPC, 128, 4, H], f32, isOutput=False)
    ast_d = nc.declare_dram_parameter("ast", [BPC, 128, 4, T], f32, isOutput=False)
    att_d = nc.declare_dram_parameter("att", [BPC, 128, 4, T], f32, isOutput=False)
    ms_d = nc.declare_dram_parameter("ms", [BPC, 128, 4, DVS], bf16, isOutput=False)
    mt_d = nc.declare_dram_parameter("mt", [BPC, 128, 4, DVT], bf16, isOutput=False)
    w_d = nc.declare_dram_parameter("wb", [128, 3, H], f32, isOutput=False)
    b_d = nc.declare_dram_parameter("bb", [128, 3], f32, isOutput=False)
    probs_d = nc.declare_dram_parameter("probs", [BPC, T, NV], f32, isOutput=True)
    preds_d = nc.declare_dram_parameter("preds", [BPC, T], i32, isOutput=True)

    with tile.TileContext(nc) as tc:
        with tc.tile_pool(name="sbuf", bufs=1) as cpool, \
             tc.tile_pool(name="maps", bufs=2) as mpool, \
             tc.tile_pool(name="attn", bufs=2) as apool, \
             tc.tile_pool(name="work", bufs=2) as wpool, \
             tc.tile_pool(name="psum", bufs=8, space="PSUM") as psum:
            w_sb = cpool.tile([128, 3, H], f32, tag="w")
            b_sb = cpool.tile([128, 3], f32, tag="b")
            nc.sync.dma_start(w_sb[:], w_d[:])
            nc.sync.dma_start(b_sb[:], b_d[:])

            for b in range(BPC):
                # ---- batch input loads ----
                ms_sb = mpool.tile([128, 4, DVS], bf16, tag="ms")
                mt_sb = mpool.tile([128, 4, DVT], bf16, tag="mt")
                nc.sync.dma_start(ms_sb[:], ms_d[b])
                nc.sync.dma_start(mt_sb[:], mt_d[b])
                ast_sb = apool.tile([128, 4, T], f32, tag="ast")
                att_sb = apool.tile([128, 4, T], f32, tag="att")
                nc.sync.dma_start(ast_sb[:], ast_d[b])
                nc.sync.dma_start(att_sb[:], att_d[b])

                # hi = in-place bf16 view; lo = bf16(x - hi), one DVE op each
                as_lo = apool.tile([128, 4, T], bf16, tag="as_lo")
                at_lo = apool.tile([128, 4, T], bf16, tag="at_lo")
                nc.gpsimd.tensor_tensor(out=as_lo[:], in0=ast_sb[:], in1=_hi(ast_sb[:]),
                                        op=AL.subtract)
                nc.gpsimd.tensor_tensor(out=at_lo[:], in0=att_sb[:], in1=_hi(att_sb[:]),
                                        op=AL.subtract)
                as_hi, at_hi = _hi(ast_sb[:]), _hi(att_sb[:])
                ms_hi, mt_hi = ms_sb[:], mt_sb[:]

                hid_sb = wpool.tile([128, 4, H], f32, tag="hid")
                nc.sync.dma_start(hid_sb[:], hid_d[b])
                for ti in range(4):
                    tsl = slice(ti * 128, (ti + 1) * 128)
                    # ---- logits (GpSimd) + softmax ----
                    scratch = cpool.tile([128, H], f32, tag="scratch")
                    logits = wpool.tile([128, 3], f32, tag="logits")
                    for k in range(3):
                        nc.vector.scalar_tensor_tensor(
                            scratch[:], hid_sb[:, ti, :], 1.0, w_sb[:, k, :],
                            AL.mult, AL.mult, accum_out=logits[:, k:k + 1])
                    lb = wpool.tile([128, 3], f32, tag="lb")
                    nc.vector.tensor_add(lb[:], logits[:], b_sb[:])
                    mx = wpool.tile([128, 1], f32, tag="mx")
                    nc.vector.reduce_max(mx[:], lb[:], axis=mybir.AxisListType.X)
                    negmx = wpool.tile([128, 1], f32, tag="negmx")
                    nc.vector.tensor_scalar_mul(negmx[:], mx[:], -1.0)
                    e3 = wpool.tile([128, 3], f32, tag="e3")
                    se = wpool.tile([128, 1], f32, tag="se")
                    nc.scalar.activation(e3[:], lb[:], AF.Exp, bias=negmx[:],
                                         scale=1.0, accum_out=se[:])
                    rse = wpool.tile([128, 1], f32, tag="rse")
                    nc.vector.reciprocal(rse[:], se[:])
                    p3 = wpool.tile([128, 3], f32, tag="p3")
                    nc.vector.tensor_scalar_mul(p3[:], e3[:], rse[:, 0:1])

                    # ---- matmuls + assembly ----
                    probs_sb = wpool.tile([128, NV], f32, tag="probs")
                    nc.scalar.copy(probs_sb[:, 0:1], p3[:, 2:3])
                    for v in range(4):
                        ps = psum.tile([128, 512], f32, tag="ps")
                        for k in range(4):
                            nc.tensor.matmul(ps[:], as_hi[:, k, tsl],
                                             ms_hi[:, k, v * 512:(v + 1) * 512],
                                             start=(k == 0), stop=False)
                        for k in range(4):
                            nc.tensor.matmul(ps[:], as_lo[:, k, tsl],
                                             ms_hi[:, k, v * 512:(v + 1) * 512],
                                             start=False, stop=(k == 3))
                        nc.scalar.mul(probs_sb[:, 1 + v * 512:1 + (v + 1) * 512],
                                      ps[:], p3[:, 0:1])
                    for v in range(2):
                        ps = psum.tile([128, 512], f32, tag="ps")
                        for k in range(4):
                            nc.tensor.matmul(ps[:], at_hi[:, k, tsl],
                                             mt_hi[:, k, v * 512:(v + 1) * 512],
                                             start=(k == 0), stop=False)
                        for k in range(4):
                            nc.tensor.matmul(ps[:], at_lo[:, k, tsl],
                                             mt_hi[:, k, v * 512:(v + 1) * 512],
                                             start=False, stop=(k == 3))
                        nc.scalar.mul(probs_sb[:, 1 + DVS + v * 512:1 + DVS + (v + 1) * 512],
                                      ps[:], p3[:, 1:2])

                    # ---- argmax ----
                    max8 = wpool.tile([128, 8], f32, tag="max8")
                    idx8 = wpool.tile([128, 8], u32, tag="idx8")
                    nc.vector.max(max8[:], probs_sb[:])
                    nc.vector.max_index(idx8[:], max8[:], probs_sb[:])

                    # ---- stores ----
                    nc.scalar.dma_start(probs_d[b, tsl, :], probs_sb[:])
                    nc.scalar.dma_start(preds_d[b, tsl].bitcast(u32), idx8[:, 0:1])

    nc.compile()
    return nc


_NC = None


def kernel(hiddens, source_attentions, source_attention_maps,
           target_attentions, target_attention_maps, W, b):
    global _NC
    if _NC is None:
        _NC = _build()
    nc = _NC

    hiddens = np.asarray(hiddens, dtype=np.float32)
    As = np.asarray(source_attentions, dtype=np.float32)
    At = np.asarray(target_attentions, dtype=np.float32)
    Ms = np.asarray(source_attention_maps, dtype=np.float32)
    Mt = np.asarray(target_attention_maps, dtype=np.float32)
    W = np.asarray(W, dtype=np.float32)
    b = np.asarray(b, dtype=np.float32)

    def tile4(x):  # [B, 4*128, V] -> [B, 128, 4, V]
        Bn, R, V = x.shape
        return np.ascontiguousarray(x.reshape(Bn, 4, 128, V).transpose(0, 2, 1, 3))

    ast = tile4(As.transpose(0, 2, 1))
    att = tile4(At.transpose(0, 2, 1))
    hiddens = tile4(hiddens)
    Ms = tile4(Ms).astype(ml_dtypes.bfloat16)  # one-hot: exact in bf16
    Mt = tile4(Mt).astype(ml_dtypes.bfloat16)
    wb = np.ascontiguousarray(np.broadcast_to(W[None], (128, 3, H)))
    bb = np.ascontiguousarray(np.broadcast_to(b[None], (128, 3)))

    in_maps = []
    for c in range(N_CORES):
        sl = slice(c * BPC, (c + 1) * BPC)
        in_maps.append({
            "hid": hiddens[sl], "ast": ast[sl], "att": att[sl],
            "ms": Ms[sl], "mt": Mt[sl], "wb": wb, "bb": bb,
        })

    res = run_bass_kernel_spmd(nc, in_maps, core_ids=list(range(N_CORES)))
    probs = np.concatenate([res.results[c]["probs"] for c in range(N_CORES)], axis=0)
    preds = np.concatenate([res.results[c]["preds"] for c in range(N_CORES)], axis=0)
    return probs, preds.astype(np.int32)
